# revision 32
# baseline (speedup 1.0000x reference)
"""TRN2 Bass kernel for nn_Attention_70257075028315.

reference:
    scores = einsum('bqd,bkd->bqk', query, key)       # B=8, Nq=Nk=2048, D=512
    probs  = softmax(scores, -1)
    out    = einsum('bqk,bkd->bqd', probs, key)

Sharding: batch b -> NeuronCore b (data parallel, fully local attention).

HW model (measured via NTFF profile, PE @2.4GHz):
- Matmul moving stream is 1 col/cycle for f32r, bf16 AND fp16 (fp8 is 2).
  A [128]x[128,512] matmul is ~216ns regardless of 16/32r dtype.
- LDWEIGHTS (~97ns fp16, ~190ns f32r/fp32) is double-buffered and hidden
  under the previous matmul's stream except between short transposes.
- PE transposes issue at ~56ns (fp16, 128 cols) vs ~213ns (fp32).
Hence: everything 16-bit. fp16 over bf16 for the 10-bit mantissa
(bf16 scores are fatal: 0.27 abs score error flips argmaxes).

Per-core program (q/k: [2048, 512] fp32):
  Phase A/B (warmup): 1 MB group DMAs issued immediately (8-deep load
    ring — DMAs run concurrently on separate queues, ~190 GB/s each);
    casts fp32->fp16 (K's cast IS k_pv in natural [kk, d] layout) on DVE
    for the critical groups, gpsimd for Q g2/g3; fp16 PE-transposes ->
    kT/qT [128(d), 4(dc), 16(tile), 128]. DMA order K0, Q(tiles 0-1),
    K1-3, Q rest: S(0) chunk c only needs K group c + q tile 0, so S(0)
    chunk matmuls interleave into the K-load stream and S(0) readiness
    waits on 4.5 MB of DMA, not 8.
  Phase C (per q-tile, software-pipelined across tiles):
    S     = qT.T @ kT   fp16, 4 d-chunk-accumulated matmuls per 512-wide
            chunk, each chunk in its OWN PSUM bank tile
    max   per chunk on DVE as soon as the chunk lands; combined, negated
    p     = exp(S - max): one ACT pass per chunk, PSUM -> SBUF fp16, with
            fused per-chunk row-sum accumulation; 1/sum via DVE reciprocal
    pT    = PE-transpose of p (fp16) -> PSUM -> DVE copy to SBUF (ALL on
            DVE: routing any through ACT reorders the exp chain, ~30us)
    o     = pT.T @ k_pv  16 kk-accumulated fp16 matmuls -> PSUM [128, 512]
    out   = o * (1/rowsum) on DVE, then DMA out.
  Q-group transposes for groups 1-3 are spread one dc-quartet per q-tile
  (copies on ACT) so no tile saturates the ps_tr ring.
  Emission order per step i: T(i), S(i+1)+E(i+1), PV(i), with an explicit
  PE-queue dep keeping PV(i) after S(i+1) so PV hides the max->exp latency
  of tile i+1. PSUM: 4 banks S chunks + 3 transpose + 1 PV accum = 8.

HW notes (NTFF-measured): PE streams 1 col/cycle at 2.4 GHz for f32r,
bf16 AND fp16 alike (fp16's win over the old f32r scores is cheaper
hidden LDWEIGHTS and 4x-faster 128x128 transposes, 56 vs 213 ns).
Steady state ~8.0-8.2 us/q-tile vs a 7.86 us column-count floor; total
~162-164 us vs ~176 us for the f32r/eager-load predecessor. fp8
(2 col/cyc) fails the 2e-2 gate for both S and PV; split-compensation
restores accuracy but cancels the speedup. GPSIMD cannot touch PSUM.
"""

import numpy as np

import concourse.bass as bass
import concourse.tile as tile
import concourse.mybir as mybir
from concourse import bacc
from concourse.bass_utils import run_bass_kernel_spmd
from concourse.masks import make_identity

FP32 = mybir.dt.float32
FP32R = mybir.dt.float32r
FP16 = mybir.dt.float16
AF = mybir.ActivationFunctionType

B, NQ, NK, D = 8, 2048, 2048, 512
P = 128
NKT = NK // P   # 16 kk tiles
NQT = NQ // P   # 16 q tiles
NDC = D // P    # 4 d chunks
NCH = NK // 512  # 4 score chunks of 512


def build(score_dtype=FP16, repeat_c=1, timed=False, pv_dtype=FP16,
          kpv_bf16=False):
    """timed=True adds an int32 [1,1] input "reps": phase C re-runs in a
    dynamic For_i loop `reps` more times (0 = just the normal kernel), so one
    NEFF can measure the phase-C slope against itself."""
    nc = bacc.Bacc("TRN2", target_bir_lowering=False, debug=False)
    q_d = nc.dram_tensor("query", [NQ, D], FP32, kind="ExternalInput").ap()
    k_d = nc.dram_tensor("key", [NK, D], FP32, kind="ExternalInput").ap()
    reps_d = None
    if timed:
        reps_d = nc.dram_tensor(
            "reps", [1, 1], mybir.dt.int32, kind="ExternalInput"
        ).ap()
    out_d = nc.dram_tensor("out", [NQ, D], FP32, kind="ExternalOutput").ap()

    q_tiles_d = q_d.rearrange("(t p) d -> t p d", p=P)
    k_tiles_d = k_d.rearrange("(t p) d -> t p d", p=P)
    out_tiles_d = out_d.rearrange("(t p) d -> t p d", p=P)

    with tile.TileContext(nc) as tc:
        _body(tc, q_tiles_d, k_tiles_d, out_tiles_d, score_dtype, repeat_c,
              reps_d, pv_dtype, kpv_bf16)
    nc.compile()
    return nc


def _body(tc, q_tiles_d, k_tiles_d, out_tiles_d, score_dtype, repeat_c,
          reps_d=None, pv_dtype=FP16, kpv_bf16=False):
    from contextlib import ExitStack

    nc = tc.nc
    reps_rv = None
    if reps_d is not None:
        regs = nc.alloc_registers("reps_regs")
        nc.regs_load(regs, reps_d[0:1, 0:1])
        reps_rv = nc.snap(regs, donate=True, min_val=0, max_val=64)
    with ExitStack() as ctx:
        persist = ctx.enter_context(tc.tile_pool(name="persist", bufs=1))
        work = ctx.enter_context(tc.tile_pool(name="work", bufs=2))
        small = ctx.enter_context(tc.tile_pool(name="small", bufs=3))
        ps_s = ctx.enter_context(tc.tile_pool(name="ps_s", bufs=4, space="PSUM"))
        ps_tr = ctx.enter_context(tc.tile_pool(name="ps_tr", bufs=3, space="PSUM"))
        ps_pv = ctx.enter_context(tc.tile_pool(name="ps_pv", bufs=1, space="PSUM"))

        ident = persist.tile([P, P], FP32)
        make_identity(nc, ident[:])
        ident16 = persist.tile([P, P], score_dtype)
        nc.vector.tensor_copy(ident16[:], ident[:])
        ident_pv = ident16
        if pv_dtype is not score_dtype:
            ident_pv = persist.tile([P, P], pv_dtype)
            nc.vector.tensor_copy(ident_pv[:], ident[:])

        # Transposed operands: [d%128, d-chunk, kk-tile, 128]
        kT = persist.tile([P, NDC, NKT, P], score_dtype)
        qT = persist.tile([P, NDC, NQT, P], score_dtype)
        kpv_dt = mybir.dt.bfloat16 if kpv_bf16 else pv_dtype
        k_pv = persist.tile([P, NKT, 512 // P, P], kpv_dt)  # natural [kk, d]

        # ---- Phase A/B: load, cast to fp16, transpose ----
        # All input DMAs issue immediately (they run concurrently on
        # separate queues); a small ring would slot-block the later loads
        # behind the first casts, serializing the whole warmup.
        load = ctx.enter_context(tc.tile_pool(name="load", bufs=8))

        # fp16 natural-layout Q, cast up-front (DVE is idle during loads);
        # per-group transposes into qT are emitted lazily from phase C.
        q16 = persist.tile([P, NQT, D], score_dtype)

        loadq = ctx.enter_context(tc.tile_pool(name="loadq", bufs=2))

        def emit_load_cast(src_d, t0, nt, pv, cast=None):
            warm_cast = cast if cast is not None else nc.vector.tensor_copy
            # one DMA + one fp16 cast per group of nt tiles
            pool = load if nt == 4 else loadq
            gt = pool.tile([P, nt, D], FP32, tag=f"ld{nt}")
            nc.sync.dma_start(
                gt[:], src_d[t0 : t0 + nt].rearrange("t p d -> p t d")
            )
            if pv is not None and kpv_dt is score_dtype:
                g16 = pv[:, t0 : t0 + nt].rearrange("p t a b -> p t (a b)")
                warm_cast(g16, gt[:])
            elif pv is not None:
                g16t = pool.tile([P, nt, D], score_dtype, tag=f"ld16{nt}")
                g16 = g16t[:]
                warm_cast(g16, gt[:])
                warm_cast(
                    pv[:, t0 : t0 + nt],
                    gt[:].rearrange("p t (a b) -> p t a b", b=P),
                )
            else:
                g16 = q16[:, t0 : t0 + nt]
                warm_cast(g16, gt[:])
            return g16

        def emit_transposes(g16, dstT, t0, nt=4):
            for dc in range(NDC):
                # always the 4-wide tile (single ring; nt<4 uses a slice)
                ptr = ps_tr.tile([P, 4, P], score_dtype, tag="tr")
                for j in range(nt):
                    nc.tensor.transpose(
                        ptr[:, j, :],
                        g16[:, j, dc * P : (dc + 1) * P],
                        ident16[:],
                    )
                # gpsimd can't read PSUM; split drain between DVE and ACT
                eng = nc.vector.tensor_copy if dc % 2 == 0 else nc.scalar.copy
                eng(dstT[:, dc, t0 : t0 + nt, :], ptr[:, 0:nt, :])

        # Q transposes after group 0 are spread one dc-quartet per q-tile
        # so no single tile carries a whole group's transpose+copy load.
        q_quarters = [0] * 4

        def emit_q_quarter(g, dc):
            ptr = ps_tr.tile([P, 4, P], score_dtype, tag="tr")
            for j in range(4):
                nc.tensor.transpose(
                    ptr[:, j, :],
                    q16[:, g * 4 + j, dc * P : (dc + 1) * P],
                    ident16[:],
                )
            # ACT only: an extra DVE copy here upsets the ps_tr ring drain
            # and stalls the PE behind the chunk maxes in the DVE queue
            nc.scalar.copy(qT[:, dc, g * 4 : (g + 1) * 4, :], ptr[:])

        def ensure_q_group(i):
            g = i // 4
            while q_quarters[g] < 4:
                emit_q_quarter(g, q_quarters[g])
                q_quarters[g] += 1

        # ---- Phase C: attention over q tiles, software-pipelined ----
        def emit_S_chunk(i, c, m4, after=None):
            """One 512-wide S chunk (4 d-accumulated matmuls) + its DVE max."""
            psc = ps_s.tile([P, 512], FP32, tag="s")
            last_mm = None
            for dc in range(NDC):
                last_mm = nc.tensor.matmul(
                    psc[:],
                    lhsT=qT[:, dc, i, :],
                    rhs=kT[:, dc, c * 4 : (c + 1) * 4, :],
                    start=(dc == 0),
                    stop=(dc == NDC - 1),
                )
                if after is not None:
                    tile.add_dep_helper(
                        last_mm.ins, after.ins, False, "S-after-prev-PV"
                    )
                    after = None
            nc.vector.reduce_max(
                m4[:, c : c + 1], psc[:], axis=mybir.AxisListType.X
            )
            return psc, last_mm

        def emit_S(i, after=None):
            """S matmuls (4 separate PSUM chunk tiles) + chunk maxes + negmax."""
            chunks = []
            m4 = small.tile([P, NCH], FP32, tag="m4")
            negmax = small.tile([P, 1], FP32, tag="negmax")
            last_mm = None
            for c in range(NCH):
                psc, last_mm = emit_S_chunk(i, c, m4, after=after)
                after = None
                chunks.append(psc)
            nc.vector.reduce_max(
                negmax[:], m4[:], axis=mybir.AxisListType.X, negate=True
            )
            return chunks, negmax, last_mm

        def emit_E(i, chunks, negmax):
            """exp(S - max) per chunk -> p (fp16) + partial row-sums; 1/sum."""
            p = work.tile([P, NCH, 512], pv_dtype, tag="p")
            rs4 = small.tile([P, NCH], FP32, tag="rs4")
            rowsum = small.tile([P, 1], FP32, tag="rowsum")
            rinv = small.tile([P, 1], FP32, tag="rinv")
            for c in range(NCH):
                nc.scalar.activation(
                    p[:, c, :], chunks[c][:], AF.Exp, bias=negmax[:],
                    accum_out=rs4[:, c : c + 1],
                )
            nc.vector.reduce_sum(rowsum[:], rs4[:], axis=mybir.AxisListType.X)
            nc.vector.reciprocal(rinv[:], rowsum[:])
            return p, rinv

        def emit_T(i, p):
            """Transpose p -> pT [128(kk), 16 tiles, 128(q)] fp16."""
            pT = work.tile([P, NKT, P], pv_dtype, tag="pT")
            for g in range(4):
                ptr = ps_tr.tile([P, 4, P], pv_dtype, tag="tr")
                for j in range(4):
                    nc.tensor.transpose(
                        ptr[:, j, :],
                        p[:, g, j * P : (j + 1) * P],
                        ident_pv[:],
                    )
                # all-DVE: routing any of these through ACT (which runs the
                # exp chain) reorders the ACT queue and costs ~30us/kernel
                nc.vector.tensor_copy(pT[:, g * 4 : (g + 1) * 4, :], ptr[:])
            return pT

        def emit_PV(i, pT, rinv, after=None):
            psum_o = ps_pv.tile([P, 512], FP32, tag="pv")
            for t in range(NKT):
                mm = nc.tensor.matmul(
                    psum_o[:],
                    lhsT=pT[:, t, :],
                    rhs=k_pv[:, t],
                    start=(t == 0),
                    stop=(t == NKT - 1),
                )
                if t == 0 and after is not None:
                    # Keep PV(i) behind S(i+1) on the PE queue so PV's work
                    # hides the max->exp latency of tile i+1.
                    tile.add_dep_helper(
                        mm.ins, after.ins, False, "pv-after-next-S"
                    )
            out_sb = work.tile([P, 512], FP32, tag="out_sb")
            nc.vector.tensor_scalar_mul(out_sb[:], psum_o[:], rinv[:])
            nc.sync.dma_start(out_tiles_d[i], out_sb[:])
            return mm

        def emit_C(warm0=None):
            state = {}
            if warm0 is not None:
                state[0] = warm0
            else:
                ensure_q_group(0)
                chunks, negmax, last_mm = emit_S(0)
                state[0] = (chunks, negmax, *emit_E(0, chunks, negmax))
            for i in range(NQT):
                chunks, negmax, p, rinv = state.pop(i)
                pT = emit_T(i, p)
                g_next = i // 4 + 1
                if g_next < 4 and q_quarters[g_next] < 4:
                    emit_q_quarter(g_next, q_quarters[g_next])
                    q_quarters[g_next] += 1
                after = None
                if i + 1 < NQT:
                    ensure_q_group(i + 1)
                    s_ps, s_nm, after = emit_S(i + 1)
                    state[i + 1] = (s_ps, s_nm, *emit_E(i + 1, s_ps, s_nm))
                emit_PV(i, pT, rinv, after=after)

        # ---- Phase A/B emission, S(0) interleaved into the K-load stream
        # (S(0) chunk c only needs K group c, so the PE fills DMA-wait).
        # Q tiles 0-1 load as a small half-group before K g1-3 so S(0)/S(1)
        # readiness waits on 4.5 MB of DMA instead of 5 MB. (Splitting the
        # K loads in half as well measured ~6us WORSE: 16 issues congest
        # the sync queue and interleave the streams.) ----
        g16 = emit_load_cast(k_tiles_d, 0, 4, k_pv)
        emit_transposes(g16, kT, 0)
        gq = emit_load_cast(q_tiles_d, 0, 2, None)
        emit_transposes(gq, qT, 0, nt=2)
        m4_0 = small.tile([P, NCH], FP32, tag="m4")
        negmax_0 = small.tile([P, 1], FP32, tag="negmax")
        chunks0 = [emit_S_chunk(0, 0, m4_0)[0]]
        for g in (1, 2, 3):
            gk = emit_load_cast(k_tiles_d, g * 4, 4, k_pv)
            emit_transposes(gk, kT, g * 4)
            chunks0.append(emit_S_chunk(0, g, m4_0)[0])
        nc.vector.reduce_max(
            negmax_0[:], m4_0[:], axis=mybir.AxisListType.X, negate=True
        )
        warm0 = (chunks0, negmax_0, *emit_E(0, chunks0, negmax_0))
        gq = emit_load_cast(q_tiles_d, 2, 2, None)
        emit_transposes(gq, qT, 2, nt=2)
        q_quarters[0] = 4
        # Q1 cast on DVE (needed earliest); Q2/Q3 on idle gpsimd (slow
        # there, ~8us each, but far off the critical path) to keep DVE
        # free for phase C maxes.
        emit_load_cast(q_tiles_d, 4, 4, None)
        for g in (2, 3):
            emit_load_cast(q_tiles_d, g * 4, 4, None, cast=nc.gpsimd.tensor_copy)

        for r in range(repeat_c):
            emit_C(warm0 if r == 0 else None)
            warm0 = None

        if reps_rv is not None:
            with tc.For_i(0, reps_rv, 1):
                emit_C()


_NC_CACHE = {}


def _get_nc(score_dtype=FP16, repeat_c=1):
    key = (str(score_dtype), repeat_c)
    if key not in _NC_CACHE:
        _NC_CACHE[key] = build(score_dtype, repeat_c)
    return _NC_CACHE[key]


def kernel(query: np.ndarray, key: np.ndarray) -> np.ndarray:
    query = np.asarray(query, dtype=np.float32)
    key = np.asarray(key, dtype=np.float32)
    assert query.shape == (B, NQ, D) and key.shape == (B, NK, D)
    nc = _get_nc()
    in_maps = [{"query": query[b], "key": key[b]} for b in range(B)]
    res = run_bass_kernel_spmd(nc, in_maps, list(range(B)))
    return np.stack([res.results[b]["out"] for b in range(B)], axis=0)


# revision 40
# speedup vs baseline: 1.0066x; 1.0066x over previous
"""TRN2 Bass kernel for nn_Attention_70257075028315.

reference:
    scores = einsum('bqd,bkd->bqk', query, key)       # B=8, Nq=Nk=2048, D=512
    probs  = softmax(scores, -1)
    out    = einsum('bqk,bkd->bqd', probs, key)

Sharding: batch b -> NeuronCore b (data parallel, fully local attention).

HW model (measured via NTFF profile, PE @2.4GHz):
- Matmul moving stream is 1 col/cycle for f32r, bf16 AND fp16 (fp8 is 2).
  A [128]x[128,512] matmul is ~216ns regardless of 16/32r dtype.
- LDWEIGHTS (~97ns fp16, ~190ns f32r/fp32) is double-buffered and hidden
  under the previous matmul's stream except between short transposes.
- PE transposes issue at ~56ns (fp16, 128 cols) vs ~213ns (fp32).
Hence: everything 16-bit. fp16 over bf16 for the 10-bit mantissa
(bf16 scores are fatal: 0.27 abs score error flips argmaxes).

Per-core program (q/k: [2048, 512] fp32):
  Phase A/B (warmup): 1 MB group DMAs issued immediately (8-deep load
    ring — DMAs run concurrently on separate queues, ~190 GB/s each);
    casts fp32->fp16 (K's cast IS k_pv in natural [kk, d] layout) on DVE
    for the critical groups, gpsimd for Q g2/g3; fp16 PE-transposes ->
    kT/qT [128(d), 4(dc), 16(tile), 128]. DMA order K0, Q(tiles 0-1),
    K1-3, Q rest: S(0) chunk c only needs K group c + q tile 0, so S(0)
    chunk matmuls interleave into the K-load stream and S(0) readiness
    waits on 4.5 MB of DMA, not 8.
  Phase C (per q-tile, software-pipelined across tiles):
    S     = qT.T @ kT   fp16, 4 d-chunk-accumulated matmuls per 512-wide
            chunk, each chunk in its OWN PSUM bank tile
    max   per chunk on DVE as soon as the chunk lands; combined, negated
    p     = exp(S - max): one ACT pass per chunk, PSUM -> SBUF fp16, with
            fused per-chunk row-sum accumulation; 1/sum via DVE reciprocal
    pT    = PE-transpose of p (fp16) -> PSUM -> DVE copy to SBUF (ALL on
            DVE: routing any through ACT reorders the exp chain, ~30us)
    o     = pT.T @ k_pv  16 kk-accumulated fp16 matmuls -> PSUM [128, 512]
    out   = o * (1/rowsum) on DVE, then DMA out.
  Q-group transposes for groups 1-3 are spread one dc-quartet per q-tile
  (copies on ACT) so no tile saturates the ps_tr ring.
  Emission order per step i: T(i), S(i+1)+E(i+1), PV(i), with an explicit
  PE-queue dep keeping PV(i) after S(i+1) so PV hides the max->exp latency
  of tile i+1. PSUM: 4 banks S chunks + 3 transpose + 1 PV accum = 8.

HW notes (NTFF-measured): PE streams 1 col/cycle at 2.4 GHz for f32r,
bf16 AND fp16 alike (fp16's win over the old f32r scores is cheaper
hidden LDWEIGHTS and 4x-faster 128x128 transposes, 56 vs 213 ns).
Steady state ~8.0-8.2 us/q-tile vs a 7.86 us column-count floor; total
~162-164 us vs ~176 us for the f32r/eager-load predecessor. fp8
(2 col/cyc) fails the 2e-2 gate for both S and PV; split-compensation
restores accuracy but cancels the speedup. GPSIMD cannot touch PSUM.
"""

import numpy as np

import concourse.bass as bass
import concourse.tile as tile
import concourse.mybir as mybir
from concourse import bacc
from concourse.bass_utils import run_bass_kernel_spmd
from concourse.masks import make_identity

FP32 = mybir.dt.float32
FP32R = mybir.dt.float32r
FP16 = mybir.dt.float16
AF = mybir.ActivationFunctionType

B, NQ, NK, D = 8, 2048, 2048, 512
P = 128
NKT = NK // P   # 16 kk tiles
NQT = NQ // P   # 16 q tiles
NDC = D // P    # 4 d chunks
NCH = NK // 512  # 4 score chunks of 512


def build(score_dtype=FP16, repeat_c=1, timed=False, pv_dtype=FP16,
          kpv_bf16=False):
    """timed=True adds an int32 [1,1] input "reps": phase C re-runs in a
    dynamic For_i loop `reps` more times (0 = just the normal kernel), so one
    NEFF can measure the phase-C slope against itself."""
    nc = bacc.Bacc("TRN2", target_bir_lowering=False, debug=False)
    q_d = nc.dram_tensor("query", [NQ, D], FP32, kind="ExternalInput").ap()
    k_d = nc.dram_tensor("key", [NK, D], FP32, kind="ExternalInput").ap()
    reps_d = None
    if timed:
        reps_d = nc.dram_tensor(
            "reps", [1, 1], mybir.dt.int32, kind="ExternalInput"
        ).ap()
    out_d = nc.dram_tensor("out", [NQ, D], FP32, kind="ExternalOutput").ap()

    q_tiles_d = q_d.rearrange("(t p) d -> t p d", p=P)
    k_tiles_d = k_d.rearrange("(t p) d -> t p d", p=P)
    out_tiles_d = out_d.rearrange("(t p) d -> t p d", p=P)
    out_pairs_d = out_d.rearrange("(j t p) d -> j p t d", t=2, p=P)

    with tile.TileContext(nc) as tc:
        _body(tc, q_tiles_d, k_tiles_d, out_pairs_d, score_dtype, repeat_c,
              reps_d, pv_dtype, kpv_bf16)
    nc.compile()
    return nc


def _body(tc, q_tiles_d, k_tiles_d, out_pairs_d, score_dtype, repeat_c,
          reps_d=None, pv_dtype=FP16, kpv_bf16=False):
    from contextlib import ExitStack

    nc = tc.nc
    reps_rv = None
    if reps_d is not None:
        regs = nc.alloc_registers("reps_regs")
        nc.regs_load(regs, reps_d[0:1, 0:1])
        reps_rv = nc.snap(regs, donate=True, min_val=0, max_val=64)
    with ExitStack() as ctx:
        persist = ctx.enter_context(tc.tile_pool(name="persist", bufs=1))
        work = ctx.enter_context(tc.tile_pool(name="work", bufs=2))
        small = ctx.enter_context(tc.tile_pool(name="small", bufs=3))
        ps_s = ctx.enter_context(tc.tile_pool(name="ps_s", bufs=4, space="PSUM"))
        ps_tr = ctx.enter_context(tc.tile_pool(name="ps_tr", bufs=3, space="PSUM"))
        ps_pv = ctx.enter_context(tc.tile_pool(name="ps_pv", bufs=1, space="PSUM"))

        ident = persist.tile([P, P], FP32)
        make_identity(nc, ident[:])
        ident16 = persist.tile([P, P], score_dtype)
        nc.vector.tensor_copy(ident16[:], ident[:])
        ident_pv = ident16
        if pv_dtype is not score_dtype:
            ident_pv = persist.tile([P, P], pv_dtype)
            nc.vector.tensor_copy(ident_pv[:], ident[:])

        # Transposed operands: [d%128, d-chunk, kk-tile, 128]
        kT = persist.tile([P, NDC, NKT, P], score_dtype)
        qT = persist.tile([P, NDC, NQT, P], score_dtype)
        kpv_dt = mybir.dt.bfloat16 if kpv_bf16 else pv_dtype
        k_pv = persist.tile([P, NKT, 512 // P, P], kpv_dt)  # natural [kk, d]

        # ---- Phase A/B: load, cast to fp16, transpose ----
        # All input DMAs issue immediately (they run concurrently on
        # separate queues); a small ring would slot-block the later loads
        # behind the first casts, serializing the whole warmup.
        load = ctx.enter_context(tc.tile_pool(name="load", bufs=8))

        # fp16 natural-layout Q, cast up-front (DVE is idle during loads);
        # per-group transposes into qT are emitted lazily from phase C.
        q16 = persist.tile([P, NQT, D], score_dtype)

        loadq = ctx.enter_context(tc.tile_pool(name="loadq", bufs=2))

        def emit_load_cast(src_d, t0, nt, pv, cast=None):
            warm_cast = cast if cast is not None else nc.vector.tensor_copy
            # one DMA + one fp16 cast per group of nt tiles
            pool = load if nt == 4 else loadq
            gt = pool.tile([P, nt, D], FP32, tag=f"ld{nt}")
            nc.sync.dma_start(
                gt[:], src_d[t0 : t0 + nt].rearrange("t p d -> p t d")
            )
            if pv is not None and kpv_dt is score_dtype:
                g16 = pv[:, t0 : t0 + nt].rearrange("p t a b -> p t (a b)")
                warm_cast(g16, gt[:])
            elif pv is not None:
                g16t = pool.tile([P, nt, D], score_dtype, tag=f"ld16{nt}")
                g16 = g16t[:]
                warm_cast(g16, gt[:])
                warm_cast(
                    pv[:, t0 : t0 + nt],
                    gt[:].rearrange("p t (a b) -> p t a b", b=P),
                )
            else:
                g16 = q16[:, t0 : t0 + nt]
                warm_cast(g16, gt[:])
            return g16

        def emit_transposes(g16, dstT, t0, nt=4):
            for dc in range(NDC):
                # always the 4-wide tile (single ring; nt<4 uses a slice)
                ptr = ps_tr.tile([P, 4, P], score_dtype, tag="tr")
                for j in range(nt):
                    nc.tensor.transpose(
                        ptr[:, j, :],
                        g16[:, j, dc * P : (dc + 1) * P],
                        ident16[:],
                    )
                # gpsimd can't read PSUM; split drain between DVE and ACT
                eng = nc.vector.tensor_copy if dc % 2 == 0 else nc.scalar.copy
                eng(dstT[:, dc, t0 : t0 + nt, :], ptr[:, 0:nt, :])

        # Q transposes after group 0 are spread one dc-quartet per q-tile
        # so no single tile carries a whole group's transpose+copy load.
        q_quarters = [0] * 4

        def emit_q_quarter(g, dc):
            ptr = ps_tr.tile([P, 4, P], score_dtype, tag="tr")
            for j in range(4):
                nc.tensor.transpose(
                    ptr[:, j, :],
                    q16[:, g * 4 + j, dc * P : (dc + 1) * P],
                    ident16[:],
                )
            # ACT only: an extra DVE copy here upsets the ps_tr ring drain
            # and stalls the PE behind the chunk maxes in the DVE queue
            nc.scalar.copy(qT[:, dc, g * 4 : (g + 1) * 4, :], ptr[:])

        def ensure_q_group(i):
            g = i // 4
            while q_quarters[g] < 4:
                emit_q_quarter(g, q_quarters[g])
                q_quarters[g] += 1

        # ---- Phase C: attention over q tiles, software-pipelined ----
        def emit_S_chunk(i, c, m4, after=None):
            """One 512-wide S chunk (4 d-accumulated matmuls) + its DVE max."""
            psc = ps_s.tile([P, 512], FP32, tag="s")
            last_mm = None
            for dc in range(NDC):
                last_mm = nc.tensor.matmul(
                    psc[:],
                    lhsT=qT[:, dc, i, :],
                    rhs=kT[:, dc, c * 4 : (c + 1) * 4, :],
                    start=(dc == 0),
                    stop=(dc == NDC - 1),
                )
                if after is not None:
                    tile.add_dep_helper(
                        last_mm.ins, after.ins, False, "S-after-prev-PV"
                    )
                    after = None
            nc.vector.reduce_max(
                m4[:, c : c + 1], psc[:], axis=mybir.AxisListType.X
            )
            return psc, last_mm

        def emit_S(i, after=None):
            """S matmuls (4 separate PSUM chunk tiles) + chunk maxes + negmax."""
            chunks = []
            m4 = small.tile([P, NCH], FP32, tag="m4")
            negmax = small.tile([P, 1], FP32, tag="negmax")
            last_mm = None
            for c in range(NCH):
                psc, last_mm = emit_S_chunk(i, c, m4, after=after)
                after = None
                chunks.append(psc)
            nc.vector.reduce_max(
                negmax[:], m4[:], axis=mybir.AxisListType.X, negate=True
            )
            return chunks, negmax, last_mm

        def emit_E(i, chunks, negmax):
            """exp(S - max) per chunk -> p (fp16) + partial row-sums; 1/sum."""
            p = work.tile([P, NCH, 512], pv_dtype, tag="p")
            rs4 = small.tile([P, NCH], FP32, tag="rs4")
            rowsum = small.tile([P, 1], FP32, tag="rowsum")
            rinv = small.tile([P, 1], FP32, tag="rinv")
            for c in range(NCH):
                nc.scalar.activation(
                    p[:, c, :], chunks[c][:], AF.Exp, bias=negmax[:],
                    accum_out=rs4[:, c : c + 1],
                )
            nc.vector.reduce_sum(rowsum[:], rs4[:], axis=mybir.AxisListType.X)
            nc.vector.reciprocal(rinv[:], rowsum[:])
            return p, rinv

        def emit_T(i, p):
            """Transpose p -> pT [128(kk), 16 tiles, 128(q)] fp16."""
            pT = work.tile([P, NKT, P], pv_dtype, tag="pT")
            for g in range(4):
                ptr = ps_tr.tile([P, 4, P], pv_dtype, tag="tr")
                for j in range(4):
                    nc.tensor.transpose(
                        ptr[:, j, :],
                        p[:, g, j * P : (j + 1) * P],
                        ident_pv[:],
                    )
                # all-DVE: routing any of these through ACT (which runs the
                # exp chain) reorders the ACT queue and costs ~30us/kernel
                nc.vector.tensor_copy(pT[:, g * 4 : (g + 1) * 4, :], ptr[:])
            return pT

        def emit_PV(i, pT, rinv, after=None):
            psum_o = ps_pv.tile([P, 512], FP32, tag="pv")
            for t in range(NKT):
                mm = nc.tensor.matmul(
                    psum_o[:],
                    lhsT=pT[:, t, :],
                    rhs=k_pv[:, t],
                    start=(t == 0),
                    stop=(t == NKT - 1),
                )
                if t == 0 and after is not None:
                    # Keep PV(i) behind S(i+1) on the PE queue so PV's work
                    # hides the max->exp latency of tile i+1.
                    tile.add_dep_helper(
                        mm.ins, after.ins, False, "pv-after-next-S"
                    )
            # one out DMA per tile (pairing two tiles per DMA to halve the
            # semaphore count measured ~1us WORSE - it delays the even
            # tile's store and couples the scales)
            out_sb = work.tile([P, 512], FP32, tag="out_sb")
            nc.vector.tensor_scalar_mul(out_sb[:], psum_o[:], rinv[:])
            nc.sync.dma_start(
                out_pairs_d[i // 2, :, i % 2, :], out_sb[:]
            )
            return mm

        def emit_C(warm0=None):
            state = {}
            if warm0 is not None:
                state[0] = warm0
            else:
                ensure_q_group(0)
                chunks, negmax, last_mm = emit_S(0)
                state[0] = (chunks, negmax, *emit_E(0, chunks, negmax))
            for i in range(NQT):
                chunks, negmax, p, rinv = state.pop(i)
                pT = emit_T(i, p)
                g_next = i // 4 + 1
                if g_next < 4 and q_quarters[g_next] < 4:
                    emit_q_quarter(g_next, q_quarters[g_next])
                    q_quarters[g_next] += 1
                after = None
                if i + 1 < NQT:
                    ensure_q_group(i + 1)
                    s_ps, s_nm, after = emit_S(i + 1)
                    state[i + 1] = (s_ps, s_nm, *emit_E(i + 1, s_ps, s_nm))
                emit_PV(i, pT, rinv, after=after)

        # ---- Phase A/B emission, S(0) interleaved into the K-load stream
        # (S(0) chunk c only needs K group c, so the PE fills DMA-wait).
        # Q tiles 0-1 load as a small half-group before K g1-3 so S(0)/S(1)
        # readiness waits on 4.5 MB of DMA instead of 5 MB. (Splitting the
        # K loads in half as well measured ~6us WORSE: 16 issues congest
        # the sync queue and interleave the streams.) ----
        g16 = emit_load_cast(k_tiles_d, 0, 4, k_pv)
        emit_transposes(g16, kT, 0)
        gq = emit_load_cast(q_tiles_d, 0, 2, None)
        emit_transposes(gq, qT, 0, nt=2)
        m4_0 = small.tile([P, NCH], FP32, tag="m4")
        negmax_0 = small.tile([P, 1], FP32, tag="negmax")
        chunks0 = [emit_S_chunk(0, 0, m4_0)[0]]
        for g in (1, 2, 3):
            gk = emit_load_cast(k_tiles_d, g * 4, 4, k_pv)
            emit_transposes(gk, kT, g * 4)
            chunks0.append(emit_S_chunk(0, g, m4_0)[0])
        nc.vector.reduce_max(
            negmax_0[:], m4_0[:], axis=mybir.AxisListType.X, negate=True
        )
        warm0 = (chunks0, negmax_0, *emit_E(0, chunks0, negmax_0))
        gq = emit_load_cast(q_tiles_d, 2, 2, None)
        emit_transposes(gq, qT, 2, nt=2)
        q_quarters[0] = 4
        # Q1 cast on DVE (needed earliest); Q2/Q3 on idle gpsimd (slow
        # there, ~8us each, but far off the critical path) to keep DVE
        # free for phase C maxes.
        emit_load_cast(q_tiles_d, 4, 4, None)
        for g in (2, 3):
            emit_load_cast(q_tiles_d, g * 4, 4, None, cast=nc.gpsimd.tensor_copy)

        for r in range(repeat_c):
            emit_C(warm0 if r == 0 else None)
            warm0 = None

        if reps_rv is not None:
            with tc.For_i(0, reps_rv, 1):
                emit_C()


_NC_CACHE = {}


def _get_nc(score_dtype=FP16, repeat_c=1):
    key = (str(score_dtype), repeat_c)
    if key not in _NC_CACHE:
        _NC_CACHE[key] = build(score_dtype, repeat_c)
    return _NC_CACHE[key]


def kernel(query: np.ndarray, key: np.ndarray) -> np.ndarray:
    query = np.asarray(query, dtype=np.float32)
    key = np.asarray(key, dtype=np.float32)
    assert query.shape == (B, NQ, D) and key.shape == (B, NK, D)
    nc = _get_nc()
    in_maps = [{"query": query[b], "key": key[b]} for b in range(B)]
    res = run_bass_kernel_spmd(nc, in_maps, list(range(B)))
    return np.stack([res.results[b]["out"] for b in range(B)], axis=0)
